# revision 4
# baseline (speedup 1.0000x reference)
"""Greedy-decode LSTM (nn_DecoderLSTM) Trainium2 Bass kernel, 8-way sharded.

Problem: T=32-step greedy decode. Per step: embed prev token, fused 4-gate
LSTM cell (H=1024), vocab projection (V=32000), argmax feeds next step.
Output: logits [T=32, B=64, V=32000] fp32.

Sharding (8 cores):
- LSTM cell: hidden dim sharded. Core j owns hidden units [128j, 128j+128):
  it holds the gate-column shards of Wh ([1024, 512] = f|i|g|o each 128 wide)
  and computes its c/h shard. h shards are AllGather'd each step (32KB).
- Vocab projection: Wout column-sharded ([1024, 4000] per core, resident in
  SBUF). Per-core top-1 (value, index) pairs are AllGather'd (512B) and every
  core computes the identical global argmax.
- emb @ Wx + bx + bh is folded on the host into one gathered table
  EW = embedding @ Wx + b [32000, 4096], column-sharded like Wh. The per-step
  embedding lookup becomes an indirect-DMA row gather of EW.

Everything is fp32: the greedy argmax chain must match the fp32 reference
exactly (token flips cascade); reduced-precision matmul modes (bf16/FP22)
perturb logits far more than the typical top-1/top-2 gap.
"""

import os
import sys
import time

for _p in ("/opt/trn_rl_repo", "/root/.axon_site/_ro/trn_rl_repo"):
    if os.path.isdir(_p) and _p not in sys.path:
        sys.path.insert(0, _p)

import numpy as np

import concourse.bass as bass
import concourse.mybir as mybir
from concourse import bacc, tile
from concourse.bass_utils import run_bass_kernel_spmd

F32 = mybir.dt.float32
I32 = mybir.dt.int32
U32 = mybir.dt.uint32

V, E, H, B, T = 32000, 512, 1024, 64, 32
NC = 8          # cores
HS = H // NC    # hidden shard = 128
VS = V // NC    # vocab shard = 4000
GS = 4 * HS     # gate-column shard = 512
KT = H // 128   # K tiles over hidden = 8
NCHUNK = 8      # vocab chunks per core
CH = VS // NCHUNK  # 500 columns per chunk
BIG = 65536.0   # index-packing offset (all idx-BIG exactly representable)


def _build(T_steps=T):
    nc = bacc.Bacc("TRN2", target_bir_lowering=False, debug=False, num_devices=NC)

    ew = nc.dram_tensor("ew", [V, GS], F32, kind="ExternalInput")
    whj = nc.dram_tensor("whj", [H, GS], F32, kind="ExternalInput")
    woutj = nc.dram_tensor("woutj", [H, VS], F32, kind="ExternalInput")
    boutb = nc.dram_tensor("boutb", [B, VS], F32, kind="ExternalInput")
    gates0 = nc.dram_tensor("gates0", [B, GS], F32, kind="ExternalInput")
    h0t = nc.dram_tensor("h0t", [H, B], F32, kind="ExternalInput")
    c0j = nc.dram_tensor("c0j", [B, HS], F32, kind="ExternalInput")
    cbase = nc.dram_tensor("cbase", [B, NCHUNK], F32, kind="ExternalInput")
    ident = nc.dram_tensor("ident", [B, B], F32, kind="ExternalInput")
    outj = nc.dram_tensor("outj", [T_steps, B, VS], F32, kind="ExternalOutput")

    Sig = mybir.ActivationFunctionType.Sigmoid
    Tanh = mybir.ActivationFunctionType.Tanh
    Op = mybir.AluOpType
    X = mybir.AxisListType.X

    with tile.TileContext(nc) as tc:
        with (
            tc.tile_pool(name="wpool", bufs=1) as wp,
            tc.tile_pool(name="state", bufs=1) as st,
            tc.tile_pool(name="work", bufs=2) as wk,
            tc.tile_pool(name="lwork", bufs=3) as lw,
            tc.tile_pool(name="gpsum", bufs=1, space="PSUM") as gp_pool,
            tc.tile_pool(name="tpsum", bufs=1, space="PSUM") as tp_pool,
            tc.tile_pool(name="lpsum", bufs=4, space="PSUM") as lp_pool,
            tc.tile_pool(name="dram", bufs=2, space="DRAM") as dr,
        ):
            # ---- resident weights/constants ----
            wh_sb = wp.tile([128, KT * GS], F32, tag="wh")
            for k in range(KT):
                nc.sync.dma_start(
                    out=wh_sb[:, k * GS:(k + 1) * GS],
                    in_=whj[k * 128:(k + 1) * 128, :],
                )
            wout_sb = wp.tile([128, KT * VS], F32, tag="wout")
            for k in range(KT):
                nc.sync.dma_start(
                    out=wout_sb[:, k * VS:(k + 1) * VS],
                    in_=woutj[k * 128:(k + 1) * 128, :],
                )
            bout_sb = wp.tile([B, VS], F32, tag="bout")
            nc.sync.dma_start(out=bout_sb[:], in_=boutb[:])
            cbase_sb = wp.tile([B, NCHUNK], F32, tag="cbase")
            nc.sync.dma_start(out=cbase_sb[:], in_=cbase[:])
            ident_sb = wp.tile([B, B], F32, tag="ident")
            nc.sync.dma_start(out=ident_sb[:], in_=ident[:])

            # ---- state ----
            hT = st.tile([128, KT * B], F32, tag="hT")  # k-tile k at cols [64k, 64k+64)
            for k in range(KT):
                nc.sync.dma_start(
                    out=hT[:, k * B:(k + 1) * B],
                    in_=h0t[k * 128:(k + 1) * 128, :],
                )
            c = st.tile([B, HS], F32, tag="c")
            nc.sync.dma_start(out=c[:], in_=c0j[:])

            gbase = wk.tile([B, GS], F32, tag="gbase")
            nc.sync.dma_start(out=gbase[:], in_=gates0[:])

            for t in range(T_steps):
                # ---- gates = gbase + h @ Wh_j  (col-tiled K pairs) ----
                gp = gp_pool.tile([128, GS], F32, tag="gp")
                for k in range(4):
                    nc.tensor.matmul(
                        gp[0:B, :],
                        lhsT=hT[:, k * B:(k + 1) * B],
                        rhs=wh_sb[:, k * GS:(k + 1) * GS],
                        start=(k == 0), stop=(k == 3),
                    )
                for k in range(4, 8):
                    nc.tensor.matmul(
                        gp[B:128, :],
                        lhsT=hT[:, k * B:(k + 1) * B],
                        rhs=wh_sb[:, k * GS:(k + 1) * GS],
                        start=(k == 4), stop=(k == 7),
                        tile_position=(0, 64),
                    )
                gsum = wk.tile([B, GS], F32, tag="gsum")
                nc.vector.tensor_tensor(gsum[:], gbase[:], gp[0:B, :], op=Op.add)
                nc.vector.tensor_tensor(gsum[:], gsum[:], gp[B:128, :], op=Op.add)

                # ---- pointwise: f,i,g,o -> c,h shard ----
                fio = wk.tile([B, GS], F32, tag="fio")  # sig(f)|sig(i)|tanh(g)|sig(o)
                nc.scalar.activation(fio[:, 0:HS], gsum[:, 0:HS], Sig)
                nc.scalar.activation(fio[:, HS:2 * HS], gsum[:, HS:2 * HS], Sig)
                nc.scalar.activation(fio[:, 2 * HS:3 * HS], gsum[:, 2 * HS:3 * HS], Tanh)
                nc.scalar.activation(fio[:, 3 * HS:4 * HS], gsum[:, 3 * HS:4 * HS], Sig)
                nc.vector.tensor_tensor(c[:], c[:], fio[:, 0:HS], op=Op.mult)
                ig = wk.tile([B, HS], F32, tag="ig")
                nc.vector.tensor_tensor(
                    ig[:], fio[:, HS:2 * HS], fio[:, 2 * HS:3 * HS], op=Op.mult
                )
                nc.vector.tensor_tensor(c[:], c[:], ig[:], op=Op.add)
                tc_ = wk.tile([B, HS], F32, tag="tc")
                nc.scalar.activation(tc_[:], c[:], Tanh)
                hj = wk.tile([B, HS], F32, tag="hj")
                nc.vector.tensor_tensor(hj[:], fio[:, 3 * HS:4 * HS], tc_[:], op=Op.mult)

                # ---- transpose h shard, AllGather full hT ----
                tp = tp_pool.tile([128, B], F32, tag="tp")
                nc.tensor.transpose(tp[:], hj[:], ident_sb[:])
                hjT = wk.tile([128, B], F32, tag="hjT")
                nc.scalar.copy(hjT[:], tp[:])

                ag1_in = dr.tile([128, B], F32, tag="ag1in")
                ag1_out = dr.tile([H, B], F32, tag="ag1out")
                nc.sync.dma_start(out=ag1_in[:], in_=hjT[:])
                nc.gpsimd.collective_compute(
                    "AllGather",
                    Op.bypass,
                    replica_groups=[list(range(NC))],
                    ins=[ag1_in[:]],
                    outs=[ag1_out[:]],
                )
                for k in range(KT):
                    nc.sync.dma_start(
                        out=hT[:, k * B:(k + 1) * B],
                        in_=ag1_out[k * 128:(k + 1) * 128, :],
                    )

                # ---- logits chunks: GEMM + bias, store, per-chunk top8 ----
                last = t == T_steps - 1
                if not last:
                    top8s = lw.tile([B, NCHUNK * 8], F32, tag="top8s")
                    idx8s = lw.tile([B, NCHUNK * 8], U32, tag="idx8s")
                for cn in range(NCHUNK):
                    # [128, 512] so the col-group slice at partition 64 stays
                    # bank-aligned (2048B/partition = one PSUM bank)
                    lp = lp_pool.tile([128, 512], F32, tag="lp")
                    for k in range(4):
                        nc.tensor.matmul(
                            lp[0:B, 0:CH],
                            lhsT=hT[:, k * B:(k + 1) * B],
                            rhs=wout_sb[:, k * VS + cn * CH:k * VS + (cn + 1) * CH],
                            start=(k == 0), stop=(k == 3),
                        )
                    for k in range(4, 8):
                        nc.tensor.matmul(
                            lp[B:128, 0:CH],
                            lhsT=hT[:, k * B:(k + 1) * B],
                            rhs=wout_sb[:, k * VS + cn * CH:k * VS + (cn + 1) * CH],
                            start=(k == 4), stop=(k == 7),
                            tile_position=(0, 64),
                        )
                    lsb = lw.tile([B, CH], F32, tag="lsb")
                    nc.vector.tensor_tensor(
                        lsb[:], bout_sb[:, cn * CH:(cn + 1) * CH], lp[0:B, 0:CH],
                        op=Op.add,
                    )
                    nc.vector.tensor_tensor(lsb[:], lsb[:], lp[B:128, 0:CH], op=Op.add)
                    nc.sync.dma_start(
                        out=outj[t, :, cn * CH:(cn + 1) * CH], in_=lsb[:]
                    )
                    if not last:
                        nc.vector.max(top8s[:, cn * 8:(cn + 1) * 8], lsb[:])
                        nc.vector.max_index(
                            idx8s[:, cn * 8:(cn + 1) * 8],
                            top8s[:, cn * 8:(cn + 1) * 8],
                            lsb[:],
                        )
                if last:
                    continue

                # ---- local top-1 across chunks ----
                vals8 = top8s[:, 0:NCHUNK * 8:8]  # [B, 8] strided view: chunk maxes
                mloc = wk.tile([B, 1], F32, tag="mloc")
                nc.vector.tensor_reduce(mloc[:], vals8, axis=X, op=Op.max)
                mask = wk.tile([B, NCHUNK], F32, tag="mask")
                nc.vector.tensor_scalar(
                    mask[:], vals8, mloc[:], None, op0=Op.is_equal
                )
                idxf = wk.tile([B, NCHUNK], F32, tag="idxf")
                nc.vector.tensor_copy(idxf[:], idx8s[:, 0:NCHUNK * 8:8])
                # global vocab index minus BIG, then min over mask hits
                nc.vector.tensor_tensor(idxf[:], idxf[:], cbase_sb[:], op=Op.add)
                nc.vector.tensor_tensor(idxf[:], idxf[:], mask[:], op=Op.mult)
                iloc = wk.tile([B, 1], F32, tag="iloc")
                nc.vector.tensor_reduce(iloc[:], idxf[:], axis=X, op=Op.min)

                # ---- AllGather (val, idx-BIG) pairs, global argmax ----
                ag2_in = dr.tile([B, 2], F32, tag="ag2in")
                ag2_out = dr.tile([NC * B, 2], F32, tag="ag2out")
                nc.sync.dma_start(out=ag2_in[:, 0:1], in_=mloc[:])
                nc.sync.dma_start(out=ag2_in[:, 1:2], in_=iloc[:])
                nc.gpsimd.collective_compute(
                    "AllGather",
                    Op.bypass,
                    replica_groups=[list(range(NC))],
                    ins=[ag2_in[:]],
                    outs=[ag2_out[:]],
                )
                gvals = wk.tile([B, NC], F32, tag="gvals")
                gidx = wk.tile([B, NC], F32, tag="gidx")
                # element (b, k) at dram row k*B+b, cols 0/1
                agv = ag2_out[:].rearrange("(k b) two -> b k two", b=B)
                nc.sync.dma_start(out=gvals[:], in_=agv[:, :, 0:1])
                nc.sync.dma_start(out=gidx[:], in_=agv[:, :, 1:2])
                gm = wk.tile([B, 1], F32, tag="gm")
                nc.vector.tensor_reduce(gm[:], gvals[:], axis=X, op=Op.max)
                gmask = wk.tile([B, NC], F32, tag="gmask")
                nc.vector.tensor_scalar(
                    gmask[:], gvals[:], gm[:], None, op0=Op.is_equal
                )
                nc.vector.tensor_tensor(gidx[:], gidx[:], gmask[:], op=Op.mult)
                tokf = wk.tile([B, 1], F32, tag="tokf")
                nc.vector.tensor_reduce(tokf[:], gidx[:], axis=X, op=Op.min)
                nc.vector.tensor_scalar(
                    tokf[:], tokf[:], float(BIG), None, op0=Op.add
                )
                toki = wk.tile([B, 1], I32, tag="toki")
                nc.vector.tensor_copy(toki[:], tokf[:])

                # ---- gather EW rows for next step ----
                gbase = wk.tile([B, GS], F32, tag="gbase")
                nc.gpsimd.indirect_dma_start(
                    out=gbase[:],
                    out_offset=None,
                    in_=ew[:],
                    in_offset=bass.IndirectOffsetOnAxis(ap=toki[:, :1], axis=0),
                )

    nc.compile()
    return nc


def _prep_inputs(embedding, Wx, bx, Wh, bh, Wout, bout, h0, c0, sos_ids, T_steps=T):
    """Host-side sharding. Returns in_maps for the 8 cores."""
    f32 = np.float32
    embedding = np.asarray(embedding, f32)
    Wx = np.asarray(Wx, f32)
    Wh = np.asarray(Wh, f32)
    Wout = np.asarray(Wout, f32)
    bias = (np.asarray(bx, f32) + np.asarray(bh, f32)).astype(f32)
    bout = np.asarray(bout, f32)
    h0 = np.asarray(h0, f32)
    c0 = np.asarray(c0, f32)
    sos = np.asarray(sos_ids).astype(np.int64)

    # EW = embedding @ Wx + (bx + bh)  [V, 4H]
    ew_full = embedding @ Wx
    ew_full += bias
    h0t_full = np.ascontiguousarray(h0.T)  # [H, B]
    ident = np.eye(B, dtype=f32)

    in_maps = []
    for j in range(NC):
        cols = np.concatenate(
            [np.arange(g * H + j * HS, g * H + (j + 1) * HS) for g in range(4)]
        )
        ewj = np.ascontiguousarray(ew_full[:, cols])
        whjv = np.ascontiguousarray(Wh[:, cols])
        woutjv = np.ascontiguousarray(Wout[:, j * VS:(j + 1) * VS])
        boutbv = np.broadcast_to(
            bout[j * VS:(j + 1) * VS], (B, VS)
        ).astype(f32)
        gates0 = np.ascontiguousarray(ewj[sos])  # [B, GS]
        c0jv = np.ascontiguousarray(c0[:, j * HS:(j + 1) * HS])
        cbase_row = np.array(
            [j * VS + cn * CH - BIG for cn in range(NCHUNK)], f32
        )
        cbasev = np.broadcast_to(cbase_row, (B, NCHUNK)).astype(f32)
        in_maps.append(
            {
                "ew": ewj,
                "whj": whjv,
                "woutj": woutjv,
                "boutb": np.ascontiguousarray(boutbv),
                "gates0": gates0,
                "h0t": h0t_full,
                "c0j": c0jv,
                "cbase": np.ascontiguousarray(cbasev),
                "ident": ident,
            }
        )
    return in_maps


_NC_CACHE = {}


def kernel(embedding, Wx, bx, Wh, bh, Wout, bout, h0, c0, sos_ids):
    in_maps = _prep_inputs(
        embedding, Wx, bx, Wh, bh, Wout, bout, h0, c0, sos_ids, T_steps=T
    )
    if T not in _NC_CACHE:
        _NC_CACHE[T] = _build(T)
    nc = _NC_CACHE[T]
    trace = bool(int(os.environ.get("KERNEL_TRACE", "0")))
    res = run_bass_kernel_spmd(
        nc, in_maps, core_ids=list(range(NC)), trace=trace
    )
    if trace:
        print(
            f"HW exec time: {res.exec_time_ns} ns "
            f"(mean {res.mean_exec_time_ns} ns, "
            f"max core {res.max_exec_time_core_id})"
        )
        kernel.last_results = res
    out = np.concatenate([r["outj"] for r in res.results], axis=-1)
    return out
